# revision 29
# baseline (speedup 1.0000x reference)
"""VQ codebook tokenizer kernel for 8 Trainium2 NeuronCores.

Problem: images [64,256,256] f32 -> patchify into 16x16 patches
         [64, 256, 256] ([B, N=256 patches, D=256]); for each patch find
         the nearest codebook row of vocab [4096,256] (squared L2);
         return (patches f32, tokens int32).

Strategy (data-parallel over batch, codebook replicated, per the hint):
 - Each of 8 cores gets 8 images = 2048 patches.
 - Patchify happens on device: strided DMAs gather each group of 128
   patches into a [128,256] SBUF tile; the tile streams back out as the
   patches output and is PE-transposed into the matmul lhsT layout.
 - v2 = sum(vocab^2) is computed on device (square + ones-matmul).
 - Device computes scores s[p,v] = <x_p,v_v> - v2_v/2 (argmax_v s =
   argmin_v d2) with bf16 matmuls accumulated in fp32 PSUM; the -v2/2
   bias rides in as a K=1 matmul from a ones row in the same PSUM
   accumulation group.
 - Reduction: DVE tensor_reduce(max) collapses each group of 16 vocab
   entries to a group max ([128,4096] -> [128,256], partly straight
   from PSUM, partly from an ACT-evicted bf16 copy); max8 + find_index
   then pick the top-8 *groups* per patch on just 256 elements.
 - Host rescores the 8x16 candidate vocab rows per patch exactly (fp64)
   and picks the argmin, reproducing the fp32 reference tokens exactly.
   This is safe: if the true argmin's group were outside the device
   top-8 groups, eight other scores would have to beat it, but the
   true argmin's margin over the 9th-best score is >1.5 while device
   noise (bf16 products, fp32 accum) is ~0.1.
"""

import numpy as np

B, H, W = 64, 256, 256
PS = 16  # patch size
V, D = 4096, 256
N_CORES = 8
B_PER_CORE = B // N_CORES                              # 8 images
PATCHES_PER_CORE = B_PER_CORE * (H // PS) * (W // PS)  # 2048
M_TILES = PATCHES_PER_CORE // 128                      # 16 tiles of 128 patches
N_CHUNK = 512                                          # one PSUM bank
PSUM_WAVE = 1024                                       # psum tile free size (2 banks)
WAVES = V // PSUM_WAVE                                 # 4
DIRECT_WAVES = 2                                       # waves reduced straight from PSUM by DVE
GROUP = 16                                             # vocab entries per reduced group
NG = V // GROUP                                        # 256 groups

_COMPILED = None      # compiled Bass program cache
LAST_RESULTS = None   # BassKernelResults of the last run (for test.py)


def build():
    import concourse.tile as tile
    from concourse import bacc, mybir
    from concourse.masks import make_identity

    nc = bacc.Bacc(trn_type="TRN2")
    f32 = mybir.dt.float32
    bf16 = mybir.dt.bfloat16
    u32 = mybir.dt.uint32

    img = nc.dram_tensor("img", [B_PER_CORE, H, W], f32, kind="ExternalInput").ap()
    vocab_t = nc.dram_tensor("vocab_t", [2, 128, V], bf16, kind="ExternalInput").ap()
    patches_out = nc.dram_tensor(
        "patches_out", [PATCHES_PER_CORE, D], f32, kind="ExternalOutput"
    ).ap()
    cand_out = nc.dram_tensor(
        "cand_out", [M_TILES, 128, 8], u32, kind="ExternalOutput"
    ).ap()

    # patch-major view: [b, gy, gx, py, px] with 64B inner runs
    img_p = img.rearrange("b (gy py) (gx px) -> b gy gx py px", py=PS, px=PS)

    gpw = PSUM_WAVE // GROUP  # groups per wave (64)
    SB_WAVES = WAVES - DIRECT_WAVES

    with tile.TileContext(nc) as tc:
        with (
            tc.tile_pool(name="const", bufs=1) as const_pool,
            tc.tile_pool(name="xtiles", bufs=6) as x_pool,
            tc.tile_pool(name="xt", bufs=4) as xt_pool,
            tc.tile_pool(name="scores", bufs=4) as s_pool,
            tc.tile_pool(name="outs", bufs=6) as o_pool,
            tc.tile_pool(name="tpsum", bufs=2, space="PSUM") as tpsum,
            tc.tile_pool(name="spsum", bufs=3, space="PSUM") as spsum,
        ):
            # --- one-time setup -------------------------------------------
            ident = const_pool.tile([128, 128], f32)
            make_identity(nc, ident)
            ones_col = const_pool.tile([128, 1], bf16)
            nc.vector.memset(ones_col[:], 1.0)

            # vocabT in per-(k, wave) tiles so the first matmuls start early
            vt = [[None, None] for _ in range(WAVES)]
            dma_engines = [nc.sync, nc.gpsimd, nc.scalar]
            for w in range(WAVES):
                for k in range(2):
                    t = const_pool.tile([128, PSUM_WAVE], bf16, tag=f"vt_{w}_{k}")
                    eng = dma_engines[(w * 2 + k) % len(dma_engines)]
                    eng.dma_start(
                        t[:], vocab_t[k, :, w * PSUM_WAVE : (w + 1) * PSUM_WAVE]
                    )
                    vt[w][k] = t

            # per-group bias: vocab is host-sorted by v2, so groups of 16
            # have near-equal v2; device computes b_g = -min_g(v2)/2 and the
            # bias is applied to the group-maxes after the reduce.
            bgrow = const_pool.tile([1, NG], bf16)
            for w in range(WAVES):
                vsq = const_pool.tile([128, 2, PSUM_WAVE], bf16, tag=f"vsq_{w}")
                nc.vector.tensor_mul(vsq[:, 0], vt[w][0][:], vt[w][0][:])
                nc.vector.tensor_mul(vsq[:, 1], vt[w][1][:], vt[w][1][:])
                for j in range(PSUM_WAVE // N_CHUNK):
                    psv = tpsum.tile([1, N_CHUNK], f32, tag="pst")
                    nc.tensor.matmul(
                        psv[:],
                        ones_col[:],
                        vsq[:, 0, j * N_CHUNK : (j + 1) * N_CHUNK],
                        start=True,
                        stop=False,
                    )
                    nc.tensor.matmul(
                        psv[:],
                        ones_col[:],
                        vsq[:, 1, j * N_CHUNK : (j + 1) * N_CHUNK],
                        start=False,
                        stop=True,
                    )
                    goff = (w * PSUM_WAVE + j * N_CHUNK) // GROUP
                    vmin = const_pool.tile([1, N_CHUNK // GROUP], f32, tag="vmin")
                    nc.vector.tensor_reduce(
                        vmin[:],
                        psv.rearrange("p (g e) -> p g e", e=GROUP),
                        mybir.AxisListType.X,
                        mybir.AluOpType.min,
                    )
                    nc.vector.tensor_scalar_mul(
                        bgrow[:, goff : goff + N_CHUNK // GROUP], vmin[:], -0.5
                    )
            bias_rep = const_pool.tile([128, NG], bf16)
            nc.gpsimd.partition_broadcast(bias_rep[:], bgrow[:])

            # --- main loop over 16 tiles of 128 patches -------------------
            for m in range(M_TILES):
                x = x_pool.tile([128, D], f32)
                bi, half = m // 2, m % 2
                for gy in range(8):
                    # [16 gx-patches, (py px)] <- [16 gx, 16 py, 16 px]
                    eng = nc.sync if gy % 2 == 0 else nc.gpsimd
                    eng.dma_start(
                        x[gy * 16 : (gy + 1) * 16, :],
                        img_p[bi, half * 8 + gy],
                    )
                (nc.sync if m % 2 else nc.gpsimd).dma_start(
                    patches_out[m * 128 : (m + 1) * 128, :], x[:]
                )

                # xt[:, k, :] = x[:, 128k:128k+128].T  (components on partitions)
                xt = xt_pool.tile([128, 2, 128], bf16)
                for k in range(2):
                    pst = tpsum.tile([128, 128], f32, tag="pst")
                    nc.tensor.transpose(pst[:], x[:, k * 128 : (k + 1) * 128], ident[:])
                    nc.scalar.activation(
                        xt[:, k], pst[:], mybir.ActivationFunctionType.Copy
                    )

                gm = o_pool.tile([128, NG], bf16, tag="gm")
                scores = s_pool.tile([128, SB_WAVES * PSUM_WAVE], bf16)
                JC = PSUM_WAVE // N_CHUNK
                for w0 in range(0, WAVES, 2):
                    pair = (w0, w0 + 1)
                    pss = {w: spsum.tile([128, PSUM_WAVE], f32, name=f"ps_{w}", tag="ps") for w in pair}
                    stationaries = [
                        (xt[:, 0], lambda w, j: vt[w][0][:, j * N_CHUNK : (j + 1) * N_CHUNK]),
                        (xt[:, 1], lambda w, j: vt[w][1][:, j * N_CHUNK : (j + 1) * N_CHUNK]),
                    ]
                    for si, (lhs, rhs_of) in enumerate(stationaries):
                        for w in pair:
                            for j in range(JC):
                                nc.tensor.matmul(
                                    pss[w][:, j * N_CHUNK : (j + 1) * N_CHUNK],
                                    lhs[:],
                                    rhs_of(w, j),
                                    start=(si == 0),
                                    stop=(si == 1),
                                )
                    for w in pair:
                        if w < SB_WAVES:
                            # evict to bf16 SBUF via ACT; group-reduce later
                            nc.scalar.activation(
                                scores[:, w * PSUM_WAVE : (w + 1) * PSUM_WAVE],
                                pss[w][:],
                                mybir.ActivationFunctionType.Copy,
                            )
                        else:
                            # group-reduce straight from PSUM on DVE
                            nc.vector.tensor_reduce(
                                gm[:, w * gpw : (w + 1) * gpw],
                                pss[w].rearrange("p (g e) -> p g e", e=GROUP),
                                mybir.AxisListType.X,
                                mybir.AluOpType.max,
                            )
                # group-reduce the SBUF-evicted waves with a pairwise
                # TT-max tree (bf16 tensor_tensor gets the 2x DVE mode,
                # tensor_reduce does not)
                ngs = SB_WAVES * gpw  # groups in the sbuf part (128)
                cur = scores
                width = GROUP
                while width > 1:
                    half = width // 2
                    v = cur.rearrange("p (g e) -> p g e", e=width)
                    if half == 1:
                        nxt = gm[:, :ngs].rearrange("p (g e) -> p g e", e=1)
                    else:
                        nxt_t = s_pool.tile([128, ngs * half], bf16, tag=f"red_{half}")
                        nxt = nxt_t.rearrange("p (g e) -> p g e", e=half)
                    nc.vector.tensor_tensor(
                        nxt, v[:, :, :half], v[:, :, half:], mybir.AluOpType.max
                    )
                    cur = nxt_t if half > 1 else None
                    width = half

                nc.vector.tensor_add(gm[:], gm[:], bias_rep[:])
                mx = o_pool.tile([128, 8], bf16)
                nc.vector.max(out=mx[:], in_=gm[:])
                mi = o_pool.tile([128, 8], u32)
                nc.vector.max_index(out=mi[:], in_max=mx[:], in_values=gm[:])
                nc.gpsimd.dma_start(cand_out[m], mi[:])

    nc.compile()
    return nc


def _sort_perm(vocab: np.ndarray) -> np.ndarray:
    v64 = vocab.astype(np.float64)
    return np.argsort((v64 * v64).sum(1), kind="stable")


def _prep_inputs(images: np.ndarray, vocab: np.ndarray):
    import ml_dtypes

    images = np.ascontiguousarray(np.asarray(images, dtype=np.float32))
    vocab = np.ascontiguousarray(np.asarray(vocab, dtype=np.float32))
    vs = vocab[_sort_perm(vocab)]     # layout prep: rows sorted by |v|^2
    vocab_t = np.ascontiguousarray(vs.T.reshape(2, 128, V).astype(ml_dtypes.bfloat16))
    return [
        {"img": images[c * B_PER_CORE : (c + 1) * B_PER_CORE], "vocab_t": vocab_t}
        for c in range(N_CORES)
    ]


def _postprocess(results, vocab: np.ndarray):
    vocab = np.asarray(vocab, dtype=np.float32)
    patches = np.empty((B, (H // PS) * (W // PS), D), dtype=np.float32)
    grp = np.empty((N_CORES * PATCHES_PER_CORE, 8), dtype=np.int64)
    for c in range(N_CORES):
        r = results[c]
        patches[c * B_PER_CORE : (c + 1) * B_PER_CORE] = r["patches_out"].reshape(
            B_PER_CORE, (H // PS) * (W // PS), D
        )
        grp[c * PATCHES_PER_CORE : (c + 1) * PATCHES_PER_CORE] = r[
            "cand_out"
        ].reshape(PATCHES_PER_CORE, 8)

    # exact rescore of the 8 candidate groups (8*GROUP vocab rows) per patch;
    # device group indices refer to the v2-sorted layout -> map back
    np.clip(grp, 0, NG - 1, out=grp)
    perm = _sort_perm(vocab)
    cand_sorted = (grp[:, :, None] * GROUP + np.arange(GROUP)[None, None, :]).reshape(
        -1, 8 * GROUP
    )                                                   # [NP, 128] sorted-space
    cand = perm[cand_sorted]                            # original vocab indices
    flat = patches.reshape(-1, D).astype(np.float64)
    v64 = vocab.astype(np.float64)
    v2 = np.sum(v64 * v64, axis=1)
    NP = flat.shape[0]
    tokens = np.empty(NP, dtype=np.int32)
    CH = 2048
    for s in range(0, NP, CH):
        e = min(s + CH, NP)
        vc = v64[cand[s:e]]                             # [ch, 128, D]
        d2 = v2[cand[s:e]] - 2.0 * np.einsum("nd,nkd->nk", flat[s:e], vc)
        # argmin with lowest-vocab-index tie-break (matches jnp.argmin);
        # candidate list is already sorted within each group block, sort fully
        order = np.argsort(cand[s:e], axis=1, kind="stable")
        d2s = np.take_along_axis(d2, order, axis=1)
        cs = np.take_along_axis(cand[s:e], order, axis=1)
        tokens[s:e] = cs[np.arange(e - s), np.argmin(d2s, axis=1)]

    return patches, tokens.reshape(B, (H // PS) * (W // PS))


def _ensure_axon_hooks():
    """run_bass_kernel_spmd(trace=True) imports antenv.axon_hooks, which this
    image lacks; provide it (with the real NTFF hook when available) so a
    BASS_TRACE=1 environment doesn't crash the run."""
    import sys
    import types

    try:
        import antenv.axon_hooks  # noqa: F401

        return
    except ImportError:
        pass
    mod = types.ModuleType("antenv.axon_hooks")
    mod._hook = None
    mod.set_axon_ntff_profile_hook = lambda h: setattr(mod, "_hook", h)
    mod.get_axon_ntff_profile_hook = lambda: mod._hook
    sys.modules["antenv.axon_hooks"] = mod
    try:
        import antenv

        antenv.axon_hooks = mod
    except ImportError:
        pass
    try:
        from trn_agent_boot.trn_boot import _ntff_profile_via_ctypes

        mod._hook = _ntff_profile_via_ctypes("/opt/axon/libaxon_pjrt.so")
    except Exception:
        pass


def kernel(images: np.ndarray, vocab: np.ndarray):
    _ensure_axon_hooks()
    from concourse.bass_utils import run_bass_kernel_spmd

    global _COMPILED, LAST_RESULTS
    if _COMPILED is None:
        _COMPILED = build()
    nc = _COMPILED

    in_maps = _prep_inputs(images, vocab)
    res = run_bass_kernel_spmd(nc, in_maps, core_ids=list(range(N_CORES)))
    LAST_RESULTS = res
    return _postprocess(res.results, vocab)


# revision 30
# speedup vs baseline: 1.0041x; 1.0041x over previous
"""VQ codebook tokenizer kernel for 8 Trainium2 NeuronCores.

Problem: images [64,256,256] f32 -> patchify into 16x16 patches
         [64, 256, 256] ([B, N=256 patches, D=256]); for each patch find
         the nearest codebook row of vocab [4096,256] (squared L2);
         return (patches f32, tokens int32).

Strategy (data-parallel over batch, codebook replicated, per the hint):
 - Each of 8 cores gets 8 images = 2048 patches.
 - Patchify happens on device: strided DMAs gather each group of 128
   patches into a [128,256] SBUF tile; the tile streams back out as the
   patches output and is PE-transposed into the matmul lhsT layout.
 - v2 = sum(vocab^2) is computed on device (square + ones-matmul).
 - Device computes scores s[p,v] = <x_p,v_v> - v2_v/2 (argmax_v s =
   argmin_v d2) with bf16 matmuls accumulated in fp32 PSUM; the -v2/2
   bias rides in as a K=1 matmul from a ones row in the same PSUM
   accumulation group.
 - Reduction: DVE tensor_reduce(max) collapses each group of 16 vocab
   entries to a group max ([128,4096] -> [128,256], partly straight
   from PSUM, partly from an ACT-evicted bf16 copy); max8 + find_index
   then pick the top-8 *groups* per patch on just 256 elements.
 - Host rescores the 8x16 candidate vocab rows per patch exactly (fp64)
   and picks the argmin, reproducing the fp32 reference tokens exactly.
   This is safe: if the true argmin's group were outside the device
   top-8 groups, eight other scores would have to beat it, but the
   true argmin's margin over the 9th-best score is >1.5 while device
   noise (bf16 products, fp32 accum) is ~0.1.
"""

import numpy as np

B, H, W = 64, 256, 256
PS = 16  # patch size
V, D = 4096, 256
N_CORES = 8
B_PER_CORE = B // N_CORES                              # 8 images
PATCHES_PER_CORE = B_PER_CORE * (H // PS) * (W // PS)  # 2048
M_TILES = PATCHES_PER_CORE // 128                      # 16 tiles of 128 patches
N_CHUNK = 512                                          # one PSUM bank
PSUM_WAVE = 1024                                       # psum tile free size (2 banks)
WAVES = V // PSUM_WAVE                                 # 4
DIRECT_WAVES = 2                                       # waves reduced straight from PSUM by DVE
GROUP = 16                                             # vocab entries per reduced group
NG = V // GROUP                                        # 256 groups

_COMPILED = None      # compiled Bass program cache
LAST_RESULTS = None   # BassKernelResults of the last run (for test.py)


def build():
    import concourse.tile as tile
    from concourse import bacc, mybir
    from concourse.masks import make_identity

    nc = bacc.Bacc(trn_type="TRN2")
    f32 = mybir.dt.float32
    bf16 = mybir.dt.bfloat16
    u32 = mybir.dt.uint32

    img = nc.dram_tensor("img", [B_PER_CORE, H, W], f32, kind="ExternalInput").ap()
    vocab_t = nc.dram_tensor("vocab_t", [2, 128, V], bf16, kind="ExternalInput").ap()
    patches_out = nc.dram_tensor(
        "patches_out", [PATCHES_PER_CORE, D], f32, kind="ExternalOutput"
    ).ap()
    cand_out = nc.dram_tensor(
        "cand_out", [M_TILES, 128, 8], u32, kind="ExternalOutput"
    ).ap()

    # patch-major view: [b, gy, gx, py, px] with 64B inner runs
    img_p = img.rearrange("b (gy py) (gx px) -> b gy gx py px", py=PS, px=PS)

    gpw = PSUM_WAVE // GROUP  # groups per wave (64)
    SB_WAVES = WAVES - DIRECT_WAVES

    with tile.TileContext(nc) as tc:
        with (
            tc.tile_pool(name="const", bufs=1) as const_pool,
            tc.tile_pool(name="xtiles", bufs=6) as x_pool,
            tc.tile_pool(name="xt", bufs=4) as xt_pool,
            tc.tile_pool(name="scores", bufs=4) as s_pool,
            tc.tile_pool(name="outs", bufs=6) as o_pool,
            tc.tile_pool(name="tpsum", bufs=2, space="PSUM") as tpsum,
            tc.tile_pool(name="spsum", bufs=3, space="PSUM") as spsum,
        ):
            # --- one-time setup -------------------------------------------
            ident = const_pool.tile([128, 128], f32)
            make_identity(nc, ident)
            ones_col = const_pool.tile([128, 1], bf16)
            nc.vector.memset(ones_col[:], 1.0)

            # vocabT in per-(k, wave) tiles so the first matmuls start early
            vt = [[None, None] for _ in range(WAVES)]
            dma_engines = [nc.sync, nc.gpsimd, nc.scalar]
            for w in range(WAVES):
                for k in range(2):
                    t = const_pool.tile([128, PSUM_WAVE], bf16, tag=f"vt_{w}_{k}")
                    eng = dma_engines[(w * 2 + k) % len(dma_engines)]
                    eng.dma_start(
                        t[:], vocab_t[k, :, w * PSUM_WAVE : (w + 1) * PSUM_WAVE]
                    )
                    vt[w][k] = t

            # per-group bias: vocab is host-sorted by v2, so groups of 16
            # have near-equal v2; device computes b_g = -min_g(v2)/2 and the
            # bias is applied to the group-maxes after the reduce.
            bgrow = const_pool.tile([1, NG], bf16)
            for w in range(WAVES):
                vsq = const_pool.tile([128, 2, PSUM_WAVE], bf16, tag=f"vsq_{w}")
                nc.vector.tensor_mul(vsq[:, 0], vt[w][0][:], vt[w][0][:])
                nc.vector.tensor_mul(vsq[:, 1], vt[w][1][:], vt[w][1][:])
                for j in range(PSUM_WAVE // N_CHUNK):
                    psv = tpsum.tile([1, N_CHUNK], f32, tag="pst")
                    nc.tensor.matmul(
                        psv[:],
                        ones_col[:],
                        vsq[:, 0, j * N_CHUNK : (j + 1) * N_CHUNK],
                        start=True,
                        stop=False,
                    )
                    nc.tensor.matmul(
                        psv[:],
                        ones_col[:],
                        vsq[:, 1, j * N_CHUNK : (j + 1) * N_CHUNK],
                        start=False,
                        stop=True,
                    )
                    goff = (w * PSUM_WAVE + j * N_CHUNK) // GROUP
                    vmin = const_pool.tile([1, N_CHUNK // GROUP], f32, tag="vmin")
                    nc.vector.tensor_reduce(
                        vmin[:],
                        psv.rearrange("p (g e) -> p g e", e=GROUP),
                        mybir.AxisListType.X,
                        mybir.AluOpType.min,
                    )
                    nc.vector.tensor_scalar_mul(
                        bgrow[:, goff : goff + N_CHUNK // GROUP], vmin[:], -0.5
                    )
            bias_rep = const_pool.tile([128, NG], bf16)
            nc.gpsimd.partition_broadcast(bias_rep[:], bgrow[:])

            # --- main loop over 16 tiles of 128 patches -------------------
            for m in range(M_TILES):
                x = x_pool.tile([128, D], f32)
                bi, half = m // 2, m % 2
                for gy in range(8):
                    # [16 gx-patches, (py px)] <- [16 gx, 16 py, 16 px]
                    eng = nc.sync if gy % 2 == 0 else nc.gpsimd
                    eng.dma_start(
                        x[gy * 16 : (gy + 1) * 16, :],
                        img_p[bi, half * 8 + gy],
                    )
                (nc.sync if m % 2 else nc.gpsimd).dma_start(
                    patches_out[m * 128 : (m + 1) * 128, :], x[:]
                )

                # xt[:, k, :] = x[:, 128k:128k+128].T  (components on partitions)
                xt = xt_pool.tile([128, 2, 128], bf16)
                for k in range(2):
                    pst = tpsum.tile([128, 128], f32, tag="pst")
                    nc.tensor.transpose(pst[:], x[:, k * 128 : (k + 1) * 128], ident[:])
                    nc.scalar.activation(
                        xt[:, k], pst[:], mybir.ActivationFunctionType.Copy
                    )

                gm = o_pool.tile([128, NG], bf16, tag="gm")
                scores = s_pool.tile([128, SB_WAVES * PSUM_WAVE], bf16)
                JC = PSUM_WAVE // N_CHUNK
                for w0 in range(0, WAVES, 2):
                    pair = (w0, w0 + 1)
                    pss = {w: spsum.tile([128, PSUM_WAVE], f32, name=f"ps_{w}", tag="ps") for w in pair}
                    stationaries = [
                        (xt[:, 0], lambda w, j: vt[w][0][:, j * N_CHUNK : (j + 1) * N_CHUNK]),
                        (xt[:, 1], lambda w, j: vt[w][1][:, j * N_CHUNK : (j + 1) * N_CHUNK]),
                    ]
                    for si, (lhs, rhs_of) in enumerate(stationaries):
                        for w in pair:
                            for j in range(JC):
                                nc.tensor.matmul(
                                    pss[w][:, j * N_CHUNK : (j + 1) * N_CHUNK],
                                    lhs[:],
                                    rhs_of(w, j),
                                    start=(si == 0),
                                    stop=(si == 1),
                                )
                    for w in pair:
                        if w < SB_WAVES:
                            # evict to bf16 SBUF via ACT; group-reduce later
                            nc.scalar.activation(
                                scores[:, w * PSUM_WAVE : (w + 1) * PSUM_WAVE],
                                pss[w][:],
                                mybir.ActivationFunctionType.Copy,
                            )
                        else:
                            # group-reduce straight from PSUM on DVE
                            nc.vector.tensor_reduce(
                                gm[:, w * gpw : (w + 1) * gpw],
                                pss[w].rearrange("p (g e) -> p g e", e=GROUP),
                                mybir.AxisListType.X,
                                mybir.AluOpType.max,
                            )
                # group-reduce the SBUF-evicted waves with a pairwise
                # TT-max tree (bf16 tensor_tensor gets the 2x DVE mode,
                # tensor_reduce does not)
                ngs = SB_WAVES * gpw  # groups in the sbuf part (128)
                cur = scores
                width = GROUP
                while width > 1:
                    half = width // 2
                    v = cur.rearrange("p (g e) -> p g e", e=width)
                    if half == 1:
                        nxt = gm[:, :ngs].rearrange("p (g e) -> p g e", e=1)
                    else:
                        nxt_t = s_pool.tile([128, ngs * half], bf16, tag=f"red_{half}")
                        nxt = nxt_t.rearrange("p (g e) -> p g e", e=half)
                    nc.vector.tensor_tensor(
                        nxt, v[:, :, :half], v[:, :, half:], mybir.AluOpType.max
                    )
                    cur = nxt_t if half > 1 else None
                    width = half

                nc.vector.tensor_add(gm[:], gm[:], bias_rep[:])
                mx = o_pool.tile([128, 8], bf16)
                nc.vector.max(out=mx[:], in_=gm[:])
                mi = o_pool.tile([128, 8], u32)
                nc.vector.max_index(out=mi[:], in_max=mx[:], in_values=gm[:])
                nc.sync.dma_start(cand_out[m], mi[:])

    nc.compile()
    return nc


def _sort_perm(vocab: np.ndarray) -> np.ndarray:
    v64 = vocab.astype(np.float64)
    return np.argsort((v64 * v64).sum(1), kind="stable")


def _prep_inputs(images: np.ndarray, vocab: np.ndarray):
    import ml_dtypes

    images = np.ascontiguousarray(np.asarray(images, dtype=np.float32))
    vocab = np.ascontiguousarray(np.asarray(vocab, dtype=np.float32))
    vs = vocab[_sort_perm(vocab)]     # layout prep: rows sorted by |v|^2
    vocab_t = np.ascontiguousarray(vs.T.reshape(2, 128, V).astype(ml_dtypes.bfloat16))
    return [
        {"img": images[c * B_PER_CORE : (c + 1) * B_PER_CORE], "vocab_t": vocab_t}
        for c in range(N_CORES)
    ]


def _postprocess(results, vocab: np.ndarray):
    vocab = np.asarray(vocab, dtype=np.float32)
    patches = np.empty((B, (H // PS) * (W // PS), D), dtype=np.float32)
    grp = np.empty((N_CORES * PATCHES_PER_CORE, 8), dtype=np.int64)
    for c in range(N_CORES):
        r = results[c]
        patches[c * B_PER_CORE : (c + 1) * B_PER_CORE] = r["patches_out"].reshape(
            B_PER_CORE, (H // PS) * (W // PS), D
        )
        grp[c * PATCHES_PER_CORE : (c + 1) * PATCHES_PER_CORE] = r[
            "cand_out"
        ].reshape(PATCHES_PER_CORE, 8)

    # exact rescore of the 8 candidate groups (8*GROUP vocab rows) per patch;
    # device group indices refer to the v2-sorted layout -> map back
    np.clip(grp, 0, NG - 1, out=grp)
    perm = _sort_perm(vocab)
    cand_sorted = (grp[:, :, None] * GROUP + np.arange(GROUP)[None, None, :]).reshape(
        -1, 8 * GROUP
    )                                                   # [NP, 128] sorted-space
    cand = perm[cand_sorted]                            # original vocab indices
    flat = patches.reshape(-1, D).astype(np.float64)
    v64 = vocab.astype(np.float64)
    v2 = np.sum(v64 * v64, axis=1)
    NP = flat.shape[0]
    tokens = np.empty(NP, dtype=np.int32)
    CH = 2048
    for s in range(0, NP, CH):
        e = min(s + CH, NP)
        vc = v64[cand[s:e]]                             # [ch, 128, D]
        d2 = v2[cand[s:e]] - 2.0 * np.einsum("nd,nkd->nk", flat[s:e], vc)
        # argmin with lowest-vocab-index tie-break (matches jnp.argmin);
        # candidate list is already sorted within each group block, sort fully
        order = np.argsort(cand[s:e], axis=1, kind="stable")
        d2s = np.take_along_axis(d2, order, axis=1)
        cs = np.take_along_axis(cand[s:e], order, axis=1)
        tokens[s:e] = cs[np.arange(e - s), np.argmin(d2s, axis=1)]

    return patches, tokens.reshape(B, (H // PS) * (W // PS))


def _ensure_axon_hooks():
    """run_bass_kernel_spmd(trace=True) imports antenv.axon_hooks, which this
    image lacks; provide it (with the real NTFF hook when available) so a
    BASS_TRACE=1 environment doesn't crash the run."""
    import sys
    import types

    try:
        import antenv.axon_hooks  # noqa: F401

        return
    except ImportError:
        pass
    mod = types.ModuleType("antenv.axon_hooks")
    mod._hook = None
    mod.set_axon_ntff_profile_hook = lambda h: setattr(mod, "_hook", h)
    mod.get_axon_ntff_profile_hook = lambda: mod._hook
    sys.modules["antenv.axon_hooks"] = mod
    try:
        import antenv

        antenv.axon_hooks = mod
    except ImportError:
        pass
    try:
        from trn_agent_boot.trn_boot import _ntff_profile_via_ctypes

        mod._hook = _ntff_profile_via_ctypes("/opt/axon/libaxon_pjrt.so")
    except Exception:
        pass


def kernel(images: np.ndarray, vocab: np.ndarray):
    _ensure_axon_hooks()
    from concourse.bass_utils import run_bass_kernel_spmd

    global _COMPILED, LAST_RESULTS
    if _COMPILED is None:
        _COMPILED = build()
    nc = _COMPILED

    in_maps = _prep_inputs(images, vocab)
    res = run_bass_kernel_spmd(nc, in_maps, core_ids=list(range(N_CORES)))
    LAST_RESULTS = res
    return _postprocess(res.results, vocab)


# revision 31
# speedup vs baseline: 1.1116x; 1.1071x over previous
"""VQ codebook tokenizer kernel for 8 Trainium2 NeuronCores.

Problem: images [64,256,256] f32 -> patchify into 16x16 patches
         [64, 256, 256] ([B, N=256 patches, D=256]); for each patch find
         the nearest codebook row of vocab [4096,256] (squared L2);
         return (patches f32, tokens int32).

Strategy (data-parallel over batch, codebook replicated, per the hint):
 - Each of 8 cores gets 8 images = 2048 patches.
 - Patchify happens on device: strided DMAs gather each group of 128
   patches into a [128,256] SBUF tile; the tile streams back out as the
   patches output and is PE-transposed into the matmul lhsT layout.
 - v2 = sum(vocab^2) is computed on device (square + ones-matmul).
 - Device computes scores s[p,v] = <x_p,v_v> - v2_v/2 (argmax_v s =
   argmin_v d2) with bf16 matmuls accumulated in fp32 PSUM; the -v2/2
   bias rides in as a K=1 matmul from a ones row in the same PSUM
   accumulation group.
 - Reduction: DVE tensor_reduce(max) collapses each group of 16 vocab
   entries to a group max ([128,4096] -> [128,256], partly straight
   from PSUM, partly from an ACT-evicted bf16 copy); max8 + find_index
   then pick the top-8 *groups* per patch on just 256 elements.
 - Host rescores the 8x16 candidate vocab rows per patch exactly (fp64)
   and picks the argmin, reproducing the fp32 reference tokens exactly.
   This is safe: if the true argmin's group were outside the device
   top-8 groups, eight other scores would have to beat it, but the
   true argmin's margin over the 9th-best score is >1.5 while device
   noise (bf16 products, fp32 accum) is ~0.1.
"""

import numpy as np

B, H, W = 64, 256, 256
PS = 16  # patch size
V, D = 4096, 256
N_CORES = 8
B_PER_CORE = B // N_CORES                              # 8 images
PATCHES_PER_CORE = B_PER_CORE * (H // PS) * (W // PS)  # 2048
M_TILES = PATCHES_PER_CORE // 128                      # 16 tiles of 128 patches
N_CHUNK = 512                                          # one PSUM bank
PSUM_WAVE = 1024                                       # psum tile free size (2 banks)
WAVES = V // PSUM_WAVE                                 # 4
DIRECT_WAVES = 2                                       # waves reduced straight from PSUM by DVE
GROUP = 16                                             # vocab entries per reduced group
NG = V // GROUP                                        # 256 groups

_COMPILED = None      # compiled Bass program cache
LAST_RESULTS = None   # BassKernelResults of the last run (for test.py)


def build():
    import concourse.tile as tile
    from concourse import bacc, mybir
    from concourse.masks import make_identity

    nc = bacc.Bacc(trn_type="TRN2")
    f32 = mybir.dt.float32
    bf16 = mybir.dt.bfloat16
    u32 = mybir.dt.uint32

    img = nc.dram_tensor("img", [B_PER_CORE, H, W], f32, kind="ExternalInput").ap()
    vocab_t = nc.dram_tensor("vocab_t", [2, 128, V], bf16, kind="ExternalInput").ap()
    patches_out = nc.dram_tensor(
        "patches_out", [PATCHES_PER_CORE, D], f32, kind="ExternalOutput"
    ).ap()
    gm_out = nc.dram_tensor(
        "gm_out", [M_TILES, 128, NG], bf16, kind="ExternalOutput"
    ).ap()

    # patch-major view: [b, gy, gx, py, px] with 64B inner runs
    img_p = img.rearrange("b (gy py) (gx px) -> b gy gx py px", py=PS, px=PS)

    gpw = PSUM_WAVE // GROUP  # groups per wave (64)
    SB_WAVES = WAVES - DIRECT_WAVES

    with tile.TileContext(nc) as tc:
        with (
            tc.tile_pool(name="const", bufs=1) as const_pool,
            tc.tile_pool(name="xtiles", bufs=6) as x_pool,
            tc.tile_pool(name="xt", bufs=4) as xt_pool,
            tc.tile_pool(name="scores", bufs=4) as s_pool,
            tc.tile_pool(name="outs", bufs=6) as o_pool,
            tc.tile_pool(name="tpsum", bufs=2, space="PSUM") as tpsum,
            tc.tile_pool(name="spsum", bufs=3, space="PSUM") as spsum,
        ):
            # --- one-time setup -------------------------------------------
            ident = const_pool.tile([128, 128], f32)
            make_identity(nc, ident)

            # vocabT in per-(k, wave) tiles so the first matmuls start early
            vt = [[None, None] for _ in range(WAVES)]
            dma_engines = [nc.sync, nc.gpsimd, nc.scalar]
            for w in range(WAVES):
                for k in range(2):
                    t = const_pool.tile([128, PSUM_WAVE], bf16, tag=f"vt_{w}_{k}")
                    eng = dma_engines[(w * 2 + k) % len(dma_engines)]
                    eng.dma_start(
                        t[:], vocab_t[k, :, w * PSUM_WAVE : (w + 1) * PSUM_WAVE]
                    )
                    vt[w][k] = t


            # --- main loop over 16 tiles of 128 patches -------------------
            for m in range(M_TILES):
                x = x_pool.tile([128, D], f32)
                bi, half = m // 2, m % 2
                for gy in range(8):
                    # [16 gx-patches, (py px)] <- [16 gx, 16 py, 16 px]
                    eng = nc.sync if gy % 2 == 0 else nc.gpsimd
                    eng.dma_start(
                        x[gy * 16 : (gy + 1) * 16, :],
                        img_p[bi, half * 8 + gy],
                    )
                (nc.sync if m % 2 else nc.gpsimd).dma_start(
                    patches_out[m * 128 : (m + 1) * 128, :], x[:]
                )

                # xt[:, k, :] = x[:, 128k:128k+128].T  (components on partitions)
                xt = xt_pool.tile([128, 2, 128], bf16)
                for k in range(2):
                    pst = tpsum.tile([128, 128], f32, tag="pst")
                    nc.tensor.transpose(pst[:], x[:, k * 128 : (k + 1) * 128], ident[:])
                    nc.scalar.activation(
                        xt[:, k], pst[:], mybir.ActivationFunctionType.Copy
                    )

                gm = o_pool.tile([128, NG], bf16, tag="gm")
                scores = s_pool.tile([128, SB_WAVES * PSUM_WAVE], bf16)
                JC = PSUM_WAVE // N_CHUNK
                for w0 in range(0, WAVES, 2):
                    pair = (w0, w0 + 1)
                    pss = {w: spsum.tile([128, PSUM_WAVE], f32, name=f"ps_{w}", tag="ps") for w in pair}
                    stationaries = [
                        (xt[:, 0], lambda w, j: vt[w][0][:, j * N_CHUNK : (j + 1) * N_CHUNK]),
                        (xt[:, 1], lambda w, j: vt[w][1][:, j * N_CHUNK : (j + 1) * N_CHUNK]),
                    ]
                    for si, (lhs, rhs_of) in enumerate(stationaries):
                        for w in pair:
                            for j in range(JC):
                                nc.tensor.matmul(
                                    pss[w][:, j * N_CHUNK : (j + 1) * N_CHUNK],
                                    lhs[:],
                                    rhs_of(w, j),
                                    start=(si == 0),
                                    stop=(si == 1),
                                )
                    for w in pair:
                        if w < SB_WAVES:
                            # evict to bf16 SBUF via ACT; group-reduce later
                            nc.scalar.activation(
                                scores[:, w * PSUM_WAVE : (w + 1) * PSUM_WAVE],
                                pss[w][:],
                                mybir.ActivationFunctionType.Copy,
                            )
                        else:
                            # group-reduce straight from PSUM on DVE
                            nc.vector.tensor_reduce(
                                gm[:, w * gpw : (w + 1) * gpw],
                                pss[w].rearrange("p (g e) -> p g e", e=GROUP),
                                mybir.AxisListType.X,
                                mybir.AluOpType.max,
                            )
                # group-reduce the SBUF-evicted waves with a pairwise
                # TT-max tree (bf16 tensor_tensor gets the 2x DVE mode,
                # tensor_reduce does not)
                ngs = SB_WAVES * gpw  # groups in the sbuf part (128)
                cur = scores
                width = GROUP
                while width > 1:
                    half = width // 2
                    v = cur.rearrange("p (g e) -> p g e", e=width)
                    if half == 1:
                        nxt = gm[:, :ngs].rearrange("p (g e) -> p g e", e=1)
                    else:
                        nxt_t = s_pool.tile([128, ngs * half], bf16, tag=f"red_{half}")
                        nxt = nxt_t.rearrange("p (g e) -> p g e", e=half)
                    nc.vector.tensor_tensor(
                        nxt, v[:, :, :half], v[:, :, half:], mybir.AluOpType.max
                    )
                    cur = nxt_t if half > 1 else None
                    width = half

                nc.sync.dma_start(gm_out[m], gm[:])

    nc.compile()
    return nc


def _sort_perm(vocab: np.ndarray) -> np.ndarray:
    v64 = vocab.astype(np.float64)
    return np.argsort((v64 * v64).sum(1), kind="stable")


def _prep_inputs(images: np.ndarray, vocab: np.ndarray):
    import ml_dtypes

    images = np.ascontiguousarray(np.asarray(images, dtype=np.float32))
    vocab = np.ascontiguousarray(np.asarray(vocab, dtype=np.float32))
    vs = vocab[_sort_perm(vocab)]     # layout prep: rows sorted by |v|^2
    vocab_t = np.ascontiguousarray(vs.T.reshape(2, 128, V).astype(ml_dtypes.bfloat16))
    return [
        {"img": images[c * B_PER_CORE : (c + 1) * B_PER_CORE], "vocab_t": vocab_t}
        for c in range(N_CORES)
    ]


def _postprocess(results, vocab: np.ndarray):
    vocab = np.asarray(vocab, dtype=np.float32)
    patches = np.empty((B, (H // PS) * (W // PS), D), dtype=np.float32)
    gm = np.empty((N_CORES * PATCHES_PER_CORE, NG), dtype=np.float32)
    for c in range(N_CORES):
        r = results[c]
        patches[c * B_PER_CORE : (c + 1) * B_PER_CORE] = r["patches_out"].reshape(
            B_PER_CORE, (H // PS) * (W // PS), D
        )
        gm[c * PATCHES_PER_CORE : (c + 1) * PATCHES_PER_CORE] = (
            r["gm_out"].reshape(PATCHES_PER_CORE, NG).astype(np.float32)
        )

    # host side of the reduce: bias the group maxes by -min_g(v2)/2 (vocab is
    # v2-sorted so the group spread is tiny) and take the top-8 groups
    perm0 = _sort_perm(vocab)
    v64s = vocab.astype(np.float64)[perm0]
    bg = -0.5 * (v64s * v64s).sum(1).reshape(NG, GROUP).min(axis=1)
    gm += bg[None, :].astype(np.float32)
    grp = np.argpartition(-gm, 8, axis=1)[:, :8].astype(np.int64)

    # exact rescore of the 8 candidate groups (8*GROUP vocab rows) per patch;
    # device group indices refer to the v2-sorted layout -> map back
    np.clip(grp, 0, NG - 1, out=grp)
    perm = _sort_perm(vocab)
    cand_sorted = (grp[:, :, None] * GROUP + np.arange(GROUP)[None, None, :]).reshape(
        -1, 8 * GROUP
    )                                                   # [NP, 128] sorted-space
    cand = perm[cand_sorted]                            # original vocab indices
    flat = patches.reshape(-1, D).astype(np.float64)
    v64 = vocab.astype(np.float64)
    v2 = np.sum(v64 * v64, axis=1)
    NP = flat.shape[0]
    tokens = np.empty(NP, dtype=np.int32)
    CH = 2048
    for s in range(0, NP, CH):
        e = min(s + CH, NP)
        vc = v64[cand[s:e]]                             # [ch, 128, D]
        d2 = v2[cand[s:e]] - 2.0 * np.einsum("nd,nkd->nk", flat[s:e], vc)
        # argmin with lowest-vocab-index tie-break (matches jnp.argmin);
        # candidate list is already sorted within each group block, sort fully
        order = np.argsort(cand[s:e], axis=1, kind="stable")
        d2s = np.take_along_axis(d2, order, axis=1)
        cs = np.take_along_axis(cand[s:e], order, axis=1)
        tokens[s:e] = cs[np.arange(e - s), np.argmin(d2s, axis=1)]

    return patches, tokens.reshape(B, (H // PS) * (W // PS))


def _ensure_axon_hooks():
    """run_bass_kernel_spmd(trace=True) imports antenv.axon_hooks, which this
    image lacks; provide it (with the real NTFF hook when available) so a
    BASS_TRACE=1 environment doesn't crash the run."""
    import sys
    import types

    try:
        import antenv.axon_hooks  # noqa: F401

        return
    except ImportError:
        pass
    mod = types.ModuleType("antenv.axon_hooks")
    mod._hook = None
    mod.set_axon_ntff_profile_hook = lambda h: setattr(mod, "_hook", h)
    mod.get_axon_ntff_profile_hook = lambda: mod._hook
    sys.modules["antenv.axon_hooks"] = mod
    try:
        import antenv

        antenv.axon_hooks = mod
    except ImportError:
        pass
    try:
        from trn_agent_boot.trn_boot import _ntff_profile_via_ctypes

        mod._hook = _ntff_profile_via_ctypes("/opt/axon/libaxon_pjrt.so")
    except Exception:
        pass


def kernel(images: np.ndarray, vocab: np.ndarray):
    _ensure_axon_hooks()
    from concourse.bass_utils import run_bass_kernel_spmd

    global _COMPILED, LAST_RESULTS
    if _COMPILED is None:
        _COMPILED = build()
    nc = _COMPILED

    in_maps = _prep_inputs(images, vocab)
    res = run_bass_kernel_spmd(nc, in_maps, core_ids=list(range(N_CORES)))
    LAST_RESULTS = res
    return _postprocess(res.results, vocab)
